# revision 15
# baseline (speedup 1.0000x reference)
"""Trainium2 Bass kernel for nn_MultiHeadAttention_39341900431503.

8-core tensor-parallel multi-head attention (B=1, S=2048, HIDDEN=2048, 16 heads,
head_dim=128). Each core computes 2 heads end-to-end (QKV proj, RoPE, causal
attention, out-proj partial); host gathers/unpermutes and sums out-proj partials.

All matmuls run in float32r (E8M11; ~1.5e-4 rel err) at full PE rate.
"""
import os
import numpy as np
from contextlib import ExitStack

import concourse.bacc as bacc
import concourse.tile as tile
from concourse.bass_types import AP
from concourse import mybir
from concourse import bass_utils

F32 = mybir.dt.float32
F32R = mybir.dt.float32r
EXP = mybir.ActivationFunctionType.Exp

B, S, HIDDEN = 1, 2048, 2048
QKV, HEADS = 2048, 16
D = 128                      # head dim
N_CORES = 8
HPC = HEADS // N_CORES       # heads per core = 2
SCALE = D ** -0.5
NEG_INF = -1e9
NT = S // 128                # 16 s/t tiles
NCH = S // 512               # 4 512-chunks

_BUILD_CACHE = {}


def _swap_ap(base, ncols):
    """AP over base[:, 0:ncols] with 64-col blocks swapped pairwise."""
    pdim = list(base.ap[0])
    nblk = ncols // 128
    return AP(base.tensor, base.offset + 64, [pdim, [128, nblk], [-64, 2], [1, 64]])


def _rep_ap(sl, nrep):
    """AP repeating a contiguous [128, F] slice nrep times along free."""
    pdim = list(sl.ap[0])
    f = sl.ap[-1][1]
    return AP(sl.tensor, sl.offset, [pdim, [0, nrep], [1, f]])


def build(mask_mode):
    """mask_mode: 'causal' | 'none' | 'full'. Returns compiled Bacc module."""
    assert mask_mode in ("causal", "none", "full")
    causal = mask_mode == "causal"
    nc = bacc.Bacc("TRN2", target_bir_lowering=False, debug=False, num_devices=N_CORES)

    # ---- DRAM I/O ----
    hT_d = nc.dram_tensor("hT", [HIDDEN, S], F32, kind="ExternalInput").ap()
    wq_d = nc.dram_tensor("wqkvT", [HIDDEN, 3 * HPC * D], F32, kind="ExternalInput").ap()
    wo_d = nc.dram_tensor("woutT", [HPC * D, HIDDEN], F32, kind="ExternalInput").ap()
    cos_d = nc.dram_tensor("cosrep", [S, 2 * D], F32, kind="ExternalInput").ap()
    sin_d = nc.dram_tensor("sinsgn", [S, 2 * D], F32, kind="ExternalInput").ap()
    sel_d = nc.dram_tensor("sel16", [16, 16], F32, kind="ExternalInput").ap()
    if causal:
        mkd_d = nc.dram_tensor("maskd", [128, 128], F32, kind="ExternalInput").ap()
        mkdT_d = nc.dram_tensor("maskdT", [128, 128], F32, kind="ExternalInput").ap()
    if mask_mode == "full":
        mk_d = nc.dram_tensor("mask", [S, S], F32, kind="ExternalInput").ap()
        mkT_d = nc.dram_tensor("maskT", [S, S], F32, kind="ExternalInput").ap()

    aw_d = nc.dram_tensor("aw", [HPC, S, S], F32, kind="ExternalOutput").ap()
    pk_d = nc.dram_tensor("pk", [S, HPC * D], F32, kind="ExternalOutput").ap()
    pv_d = nc.dram_tensor("pv", [S, HPC * D], F32, kind="ExternalOutput").ap()
    # out-proj partial, TRANSPOSED: [hid, s]; host sums cores then transposes
    op_d = nc.dram_tensor("outp", [HIDDEN, S], F32, kind="ExternalOutput").ap()
    rec_d = nc.dram_tensor("rec", [HPC, 128, NT], F32, kind="ExternalOutput").ap()

    def ncols_of(s_tile):
        return (s_tile + 1) * 128 if causal else S

    with tile.TileContext(nc) as tc:
        with ExitStack() as octx:
            # ---- persistent residents ----
            pers = octx.enter_context(tc.tile_pool(name="pers", bufs=1))
            qT = [pers.tile([128, S], F32R, tag=f"qT{h}", name=f"qT{h}") for h in range(HPC)]
            kT = [pers.tile([128, S], F32R, tag=f"kT{h}", name=f"kT{h}") for h in range(HPC)]
            v_all = pers.tile([128, NT, HPC * D], F32R, tag="v_all")
            ctxT = [pers.tile([128, S], F32R, tag=f"ctxT{h}", name=f"ctxT{h}") for h in range(HPC)]
            wo_sb = pers.tile([128, HPC, HIDDEN], F32R, tag="wo")
            recip_all = [pers.tile([128, NT], F32, tag=f"recip{h}", name=f"recip{h}") for h in range(HPC)]
            sel_sb = pers.tile([16, 16], F32R, tag="sel")
            ident = pers.tile([128, 128], F32, tag="ident")
            from concourse import masks as _masks
            _masks.make_identity(nc, ident[:])
            if causal:
                mkd = pers.tile([128, 128], F32, tag="mkd")
                mkdT = pers.tile([128, 128], F32, tag="mkdT")
                nc.sync.dma_start(mkd[:], mkd_d[:])
                nc.sync.dma_start(mkdT[:], mkdT_d[:])

            with ExitStack() as p1:
                p1sb = p1.enter_context(tc.tile_pool(name="p1sb", bufs=2))
                wpool = p1.enter_context(tc.tile_pool(name="wpool", bufs=1))
                p2sb = p1.enter_context(tc.tile_pool(name="p2sb", bufs=2))
                p2sb3 = p1.enter_context(tc.tile_pool(name="p2sb3", bufs=3))
                ptpool = p1.enter_context(tc.tile_pool(name="ptpool", bufs=4))
                mpool = p1.enter_context(tc.tile_pool(name="mpool", bufs=3))
                osb_p = p1.enter_context(tc.tile_pool(name="osbp", bufs=2))
                qkvps = p1.enter_context(tc.tile_pool(name="qkvps", bufs=2, space="PSUM"))
                p512 = p1.enter_context(tc.tile_pool(name="p512", bufs=2, space="PSUM"))
                cxops = p1.enter_context(tc.tile_pool(name="cxops", bufs=2, space="PSUM"))

                # w_qkv slices: split into per-h-tile DMAs so QKV starts early
                w_sb = wpool.tile([128, NT, 3 * HPC * D], F32R, tag="wqkv")
                for a in range(NT):
                    nc.gpsimd.dma_start(
                        w_sb[:, a, :],
                        wq_d[a * 128 : (a + 1) * 128, :],
                    )

                QW = 2 * HPC * D       # 512: q+k region width
                VW = HPC * D           # 256: v region width
                tmax_of = (lambda c: c * 4 + 3) if causal else (lambda c: NT - 1)

                def qkv_step(st):
                    s0 = st * 128
                    h_sb = p1sb.tile([128, NT, 128], F32R, tag="hT", bufs=2)
                    nc.gpsimd.dma_start(
                        h_sb[:], hT_d[:, s0 : s0 + 128].rearrange("(a p) s -> p a s", p=128)
                    )
                    qkv_ps = qkvps.tile([128, 3 * HPC * D], F32, tag="qkv")
                    for a in range(NT):
                        nc.tensor.matmul(
                            qkv_ps[:, 0:512], h_sb[:, a, :], w_sb[:, a, 0:512],
                            start=(a == 0), stop=(a == NT - 1),
                        )
                        nc.tensor.matmul(
                            qkv_ps[:, 512:768], h_sb[:, a, :], w_sb[:, a, 512:768],
                            start=(a == 0), stop=(a == NT - 1),
                        )
                    knat = p1sb.tile([128, VW], F32, tag="knat")
                    nc.any.tensor_copy(knat[:], qkv_ps[:, VW : 2 * VW])
                    nc.sync.dma_start(pk_d[s0 : s0 + 128, :], knat[:])
                    vnat = p1sb.tile([128, VW], F32, tag="vnat")
                    nc.any.tensor_copy(vnat[:], qkv_ps[:, 2 * VW : 3 * VW])
                    nc.sync.dma_start(pv_d[s0 : s0 + 128, :], vnat[:])
                    nc.vector.tensor_copy(v_all[:, st, :], qkv_ps[:, 2 * VW : 3 * VW])

                    # RoPE: roped = x*cosrep + swap(x)*sinsgn
                    base = qkv_ps[:]
                    tA = p1sb.tile([128, QW], F32, tag="ropeA")
                    tB = p1sb.tile([128, QW], F32, tag="ropeB")
                    roped = p1sb.tile([128, QW], F32, tag="roped")
                    cos_sb = p1sb.tile([128, 2 * D], F32, tag="cos")
                    sin_sb = p1sb.tile([128, 2 * D], F32, tag="sin")
                    nc.sync.dma_start(cos_sb[:], cos_d[s0 : s0 + 128, :])
                    nc.sync.dma_start(sin_sb[:], sin_d[s0 : s0 + 128, :])
                    cos_ap = _rep_ap(cos_sb[:], 2)
                    sin_ap = _rep_ap(sin_sb[:], 2)
                    nc.vector.tensor_mul(
                        tA[:].rearrange("p (a f) -> p a f", a=2, f=2 * D),
                        base[:, 0:QW].rearrange("p (a f) -> p a f", a=2, f=2 * D), cos_ap)
                    nc.vector.tensor_mul(
                        tB[:].rearrange("p (a f) -> p a f", a=2, f=2 * D),
                        _swap_ap(base, QW), sin_ap)
                    nc.vector.tensor_add(roped[:], tA[:], tB[:])
                    for j in range(2 * HPC):
                        t_ps = p512.tile([128, 128], F32, tag="p512", name=f"tr{st}_{j}")
                        nc.tensor.transpose(t_ps[:], roped[:, j * 128 : (j + 1) * 128], ident[:])
                        dst = qT[j] if j < HPC else kT[j - HPC]
                        nc.vector.tensor_copy(dst[:, s0 : s0 + 128], t_ps[:])

                def st_row(h, st):
                    # ST orientation row block: aw output + row sums
                    s0 = st * 128
                    ncols = ncols_of(st)
                    nchunk = (ncols + 511) // 512
                    pst = p2sb3.tile([128, S], F32, tag="pst")
                    sums = p2sb.tile([128, NCH], F32, tag="sums")
                    for c in range(nchunk):
                        n = min(512, ncols - c * 512)
                        pps = p512.tile([128, 512], F32, tag="p512", name=f"pps{h}_{st}_{c}")
                        nc.tensor.matmul(
                            pps[:, 0:n], qT[h][:, s0 : s0 + 128],
                            kT[h][:, c * 512 : c * 512 + n],
                            start=True, stop=True,
                        )
                        if causal and c == nchunk - 1:
                            off = ncols - 128 - c * 512
                            nc.vector.tensor_add(
                                pps[:, off : off + 128], pps[:, off : off + 128], mkd[:]
                            )
                        elif mask_mode == "full":
                            mt = mpool.tile([128, 512], F32, tag="mst")
                            nc.sync.dma_start(mt[:, 0:n], mk_d[s0 : s0 + 128, c * 512 : c * 512 + n])
                            nc.vector.tensor_add(pps[:, 0:n], pps[:, 0:n], mt[:, 0:n])
                        nc.scalar.activation(
                            pst[:, c * 512 : c * 512 + n], pps[:, 0:n], EXP,
                            scale=SCALE, accum_out=sums[:, c : c + 1],
                        )
                    tot = p2sb.tile([128, 1], F32, tag="tot")
                    if nchunk > 1:
                        nc.vector.reduce_sum(tot[:], sums[:, 0:nchunk], axis=mybir.AxisListType.X)
                    else:
                        nc.vector.tensor_copy(tot[:], sums[:, 0:1])
                    nc.vector.reciprocal(recip_all[h][:, st : st + 1], tot[:])
                    nc.sync.dma_start(aw_d[h, s0 : s0 + 128, 0:ncols], pst[:, 0:ncols])

                def t_burst(h, c):
                    # pT + AV for chunk c (software-pipelined by one t-step)
                    tmax = tmax_of(c)
                    cps = cxops.tile([128, 512], F32, tag="cx", name=f"ctx{h}_{c}")
                    pending = None
                    for t in range(tmax + 1):
                        t0 = t * 128
                        off = max(0, t0 - c * 512) if causal else 0
                        ptp = p512.tile([128, 512], F32, tag="p512", name=f"ptp{h}_{t}_{c}")
                        nc.tensor.matmul(
                            ptp[:, off:512], kT[h][:, t0 : t0 + 128],
                            qT[h][:, c * 512 + off : (c + 1) * 512],
                            start=True, stop=True,
                        )
                        if pending is not None:
                            pt, poff, ptsb = pending
                            nc.tensor.matmul(
                                cps[:, poff:512], v_all[:, pt, h * D : (h + 1) * D],
                                ptsb[:, poff:512],
                                start=(pt == 0), stop=False, skip_group_check=True,
                            )
                        if causal and t // 4 == c:
                            nc.vector.tensor_add(
                                ptp[:, off : off + 128], ptp[:, off : off + 128], mkdT[:]
                            )
                        elif mask_mode == "full":
                            mtT = mpool.tile([128, 512], F32, tag="mtT")
                            nc.sync.dma_start(mtT[:], mkT_d[t0 : t0 + 128, c * 512 : (c + 1) * 512])
                            nc.vector.tensor_add(ptp[:], ptp[:], mtT[:])
                        ptsb = ptpool.tile([128, 512], F32R, tag="ptsb", name=f"ptsb{h}_{t}_{c}")
                        nc.scalar.activation(ptsb[:, off:512], ptp[:, off:512], EXP, scale=SCALE)
                        pending = (t, off, ptsb)
                    pt, poff, ptsb = pending
                    nc.tensor.matmul(
                        cps[:, poff:512], v_all[:, pt, h * D : (h + 1) * D],
                        ptsb[:, poff:512],
                        start=(pt == 0), stop=True, skip_group_check=True,
                    )

                    # rbc for this chunk: rbc[p, s] = 1/rowsum[s]
                    rt_ps = p512.tile([16, 128], F32, tag="p512", name=f"rtps{h}_{c}")
                    nc.tensor.transpose(rt_ps[:], recip_all[h][:], ident[:])
                    rt16 = p2sb.tile([16, 128], F32R, tag="rt16")
                    nc.vector.tensor_copy(rt16[:], rt_ps[:])
                    rbc = p2sb.tile([128, 512], F32, tag="rbc")
                    for g in range(4):
                        st = 4 * c + g if causal else g  # noqa (causal always here)
                        sel_ap = AP(
                            sel_sb[:].tensor, sel_sb[:].offset + st,
                            [list(sel_sb[:].ap[0]), [0, 128]],
                        )
                        rb_ps = p512.tile([128, 128], F32, tag="p512", name=f"rbps{h}_{c}_{g}")
                        nc.tensor.matmul(
                            rb_ps[:], sel_ap, rt16[:],
                            start=True, stop=True,
                        )
                        nc.vector.tensor_copy(rbc[:, g * 128 : (g + 1) * 128], rb_ps[:])
                    nc.vector.tensor_mul(
                        ctxT[h][:, c * 512 : (c + 1) * 512], cps[:], rbc[:],
                    )

                def outproj_chunk(c):
                    # outp[hid, s-chunk] = sum_h sum_d woutT[d, hid] ctxT[d, s]
                    for ht in range(NT):
                        hh0 = ht * 128
                        o_ps = cxops.tile([128, 512], F32, tag="cx", name=f"o{ht}_{c}")
                        for h in range(HPC):
                            nc.tensor.matmul(
                                o_ps[:], wo_sb[:, h, hh0 : hh0 + 128],
                                ctxT[h][:, c * 512 : (c + 1) * 512],
                                start=(h == 0), stop=(h == HPC - 1),
                                skip_group_check=True,
                            )
                        osb = osb_p.tile([128, 512], F32, tag="osb")
                        nc.any.tensor_copy(osb[:], o_ps[:])
                        nc.sync.dma_start(
                            op_d[hh0 : hh0 + 128, c * 512 : (c + 1) * 512], osb[:]
                        )

                # ---------- fused schedule ----------
                for st in range(NT):
                    qkv_step(st)
                    if st == 0:
                        # late-load phase-2/3 weights so they don't delay QKV DMAs
                        nc.gpsimd.dma_start(wo_sb[:], wo_d.rearrange("(h p) f -> p h f", p=128))
                        nc.gpsimd.dma_start(sel_sb[:], sel_d[:])
                    for h in range(HPC):
                        st_row(h, st)
                    if causal and st % 4 == 3:
                        c = st // 4
                        for h in range(HPC):
                            t_burst(h, c)
                        outproj_chunk(c)
                if not causal:
                    for c in range(NCH):
                        for h in range(HPC):
                            t_burst(h, c)
                        outproj_chunk(c)
                for h in range(HPC):
                    nc.sync.dma_start(rec_d[h], recip_all[h][:])

    nc.compile()
    return nc


def _host_consts():
    inv_freq = 1.0 / (10000.0 ** (np.arange(0, D, 2, dtype=np.float64) / D))  # [64]
    ang = np.arange(S, dtype=np.float64)[:, None] * inv_freq[None, :]          # [S, 64]
    cos = np.cos(ang).astype(np.float32)
    sin = np.sin(ang).astype(np.float32)
    cosrep = np.tile(cos, (1, 4))                                              # [S, 256]
    sinsgn = np.concatenate([-sin, sin, -sin, sin], axis=1)                    # [S, 256]
    i = np.arange(128)
    maskd = np.where(i[:, None] >= i[None, :], 0.0, NEG_INF).astype(np.float32)
    maskdT = maskd.T.copy()
    sel16 = np.eye(16, dtype=np.float32)
    return cosrep, sinsgn, maskd, maskdT, sel16


def _detect_mode(attn_mask):
    if not np.any(attn_mask):
        return "none"
    i = np.arange(S)
    causal_ref = np.where(i[:, None] >= i[None, :], 0.0, np.float32(NEG_INF)).astype(np.float32)
    if np.array_equal(attn_mask, causal_ref):
        return "causal"
    return "full"


def kernel(hidden_states, attn_mask, w_qkv, w_out):
    hidden_states = np.ascontiguousarray(hidden_states, dtype=np.float32)
    attn_mask = np.ascontiguousarray(attn_mask, dtype=np.float32)
    w_qkv = np.ascontiguousarray(w_qkv, dtype=np.float32)
    w_out = np.ascontiguousarray(w_out, dtype=np.float32)

    mode = _detect_mode(attn_mask)
    if mode not in _BUILD_CACHE:
        _BUILD_CACHE[mode] = build(mode)
    nc = _BUILD_CACHE[mode]

    cosrep, sinsgn, maskd, maskdT, sel16 = _host_consts()
    hT = np.ascontiguousarray(hidden_states[0].T)          # [HIDDEN, S]

    in_maps = []
    for core in range(N_CORES):
        heads = [HPC * core + j for j in range(HPC)]
        d = np.arange(D)
        rows = np.concatenate(
            [base + d * HEADS + h for base in (0, QKV, 2 * QKV) for h in heads]
        )
        wqkvT = np.ascontiguousarray(w_qkv[rows, :].T)     # [HIDDEN, 768]
        cols = np.concatenate([d * HEADS + h for h in heads])
        woutT = np.ascontiguousarray(w_out[:, cols].T)     # [256, HIDDEN]
        m = {
            "hT": hT, "wqkvT": wqkvT, "woutT": woutT,
            "cosrep": cosrep, "sinsgn": sinsgn, "sel16": sel16,
        }
        if mode == "causal":
            m["maskd"] = maskd
            m["maskdT"] = maskdT
        if mode == "full":
            m["mask"] = attn_mask
            m["maskT"] = np.ascontiguousarray(attn_mask.T)
        in_maps.append(m)

    trace = bool(int(os.environ.get("KERNEL_TRACE", "0")))
    kwargs = {}
    if trace:
        import ntff_shim
        ntff_shim.install()
        kwargs = {"trace": True, "trace_cores": [0]}
    res = bass_utils.run_bass_kernel_spmd(nc, in_maps, core_ids=list(range(N_CORES)), **kwargs)
    kernel.last_exec_time_ns = res.exec_time_ns
    kernel.last_results = res

    attn_output_T = np.zeros((HIDDEN, S), dtype=np.float32)
    aw = np.empty((HEADS, S, S), dtype=np.float32)
    pk = np.empty((HEADS, S, D), dtype=np.float32)
    pv = np.empty((HEADS, S, D), dtype=np.float32)
    for core in range(N_CORES):
        r = res.results[core]
        attn_output_T += r["outp"]
        blk = aw[HPC * core : HPC * (core + 1)]
        blk[:] = r["aw"]
        rec = r["rec"]  # [HPC, 128, NT]
        for j in range(HPC):
            recip_s = rec[j].T.reshape(S)  # [st,p] -> s = st*128+p
            blk[j] *= recip_s[:, None]
        pk[HPC * core : HPC * (core + 1)] = r["pk"].reshape(S, HPC, D).transpose(1, 0, 2)
        pv[HPC * core : HPC * (core + 1)] = r["pv"].reshape(S, HPC, D).transpose(1, 0, 2)

    return (
        np.ascontiguousarray(attn_output_T.T)[None, :, :],
        aw[None, :, :, :],
        pk[None, :, :, :],
        pv[None, :, :, :],
    )


# revision 16
# speedup vs baseline: 1.0377x; 1.0377x over previous
"""Trainium2 Bass kernel for nn_MultiHeadAttention_39341900431503.

8-core tensor-parallel multi-head attention (B=1, S=2048, HIDDEN=2048, 16 heads,
head_dim=128). Each core computes 2 heads end-to-end (QKV proj, RoPE, causal
attention, out-proj partial); host gathers/unpermutes and sums out-proj partials.

All matmuls run in float32r (E8M11; ~1.5e-4 rel err) at full PE rate.
"""
import os
import numpy as np
from contextlib import ExitStack

import concourse.bacc as bacc
import concourse.tile as tile
from concourse.bass_types import AP
from concourse import mybir
from concourse import bass_utils

F32 = mybir.dt.float32
F32R = mybir.dt.float32r
EXP = mybir.ActivationFunctionType.Exp

B, S, HIDDEN = 1, 2048, 2048
QKV, HEADS = 2048, 16
D = 128                      # head dim
N_CORES = 8
HPC = HEADS // N_CORES       # heads per core = 2
SCALE = D ** -0.5
NEG_INF = -1e9
NT = S // 128                # 16 s/t tiles
NCH = S // 512               # 4 512-chunks

_BUILD_CACHE = {}


def _swap_ap(base, ncols):
    """AP over base[:, 0:ncols] with 64-col blocks swapped pairwise."""
    pdim = list(base.ap[0])
    nblk = ncols // 128
    return AP(base.tensor, base.offset + 64, [pdim, [128, nblk], [-64, 2], [1, 64]])


def _rep_ap(sl, nrep):
    """AP repeating a contiguous [128, F] slice nrep times along free."""
    pdim = list(sl.ap[0])
    f = sl.ap[-1][1]
    return AP(sl.tensor, sl.offset, [pdim, [0, nrep], [1, f]])


def build(mask_mode):
    """mask_mode: 'causal' | 'none' | 'full'. Returns compiled Bacc module."""
    assert mask_mode in ("causal", "none", "full")
    causal = mask_mode == "causal"
    nc = bacc.Bacc("TRN2", target_bir_lowering=False, debug=False, num_devices=N_CORES)

    # ---- DRAM I/O ----
    hT_d = nc.dram_tensor("hTc", [NT, 128, S], F32, kind="ExternalInput").ap()
    wq_d = nc.dram_tensor("wqkvT", [HIDDEN, 3 * HPC * D], F32, kind="ExternalInput").ap()
    wo_d = nc.dram_tensor("woutT", [HPC * D, HIDDEN], F32, kind="ExternalInput").ap()
    cos_d = nc.dram_tensor("cosrep", [S, 2 * D], F32, kind="ExternalInput").ap()
    sin_d = nc.dram_tensor("sinsgn", [S, 2 * D], F32, kind="ExternalInput").ap()
    sel_d = nc.dram_tensor("sel16", [16, 16], F32, kind="ExternalInput").ap()
    if causal:
        mkd_d = nc.dram_tensor("maskd", [128, 128], F32, kind="ExternalInput").ap()
        mkdT_d = nc.dram_tensor("maskdT", [128, 128], F32, kind="ExternalInput").ap()
    if mask_mode == "full":
        mk_d = nc.dram_tensor("mask", [S, S], F32, kind="ExternalInput").ap()
        mkT_d = nc.dram_tensor("maskT", [S, S], F32, kind="ExternalInput").ap()

    aw_d = nc.dram_tensor("aw", [HPC, S, S], F32, kind="ExternalOutput").ap()
    pk_d = nc.dram_tensor("pk", [S, HPC * D], F32, kind="ExternalOutput").ap()
    pv_d = nc.dram_tensor("pv", [S, HPC * D], F32, kind="ExternalOutput").ap()
    # out-proj partial, TRANSPOSED: [hid, s]; host sums cores then transposes
    op_d = nc.dram_tensor("outp", [HIDDEN, S], F32, kind="ExternalOutput").ap()
    rec_d = nc.dram_tensor("rec", [HPC, 128, NT], F32, kind="ExternalOutput").ap()

    def ncols_of(s_tile):
        return (s_tile + 1) * 128 if causal else S

    with tile.TileContext(nc) as tc:
        with ExitStack() as octx:
            # ---- persistent residents ----
            pers = octx.enter_context(tc.tile_pool(name="pers", bufs=1))
            qT = [pers.tile([128, S], F32R, tag=f"qT{h}", name=f"qT{h}") for h in range(HPC)]
            kT = [pers.tile([128, S], F32R, tag=f"kT{h}", name=f"kT{h}") for h in range(HPC)]
            v_all = pers.tile([128, NT, HPC * D], F32R, tag="v_all")
            ctxT = [pers.tile([128, S], F32R, tag=f"ctxT{h}", name=f"ctxT{h}") for h in range(HPC)]
            wo_sb = pers.tile([128, HPC, HIDDEN], F32R, tag="wo")
            recip_all = [pers.tile([128, NT], F32, tag=f"recip{h}", name=f"recip{h}") for h in range(HPC)]
            sel_sb = pers.tile([16, 16], F32R, tag="sel")
            ident = pers.tile([128, 128], F32, tag="ident")
            from concourse import masks as _masks
            _masks.make_identity(nc, ident[:])
            if causal:
                mkd = pers.tile([128, 128], F32, tag="mkd")
                mkdT = pers.tile([128, 128], F32, tag="mkdT")
                nc.sync.dma_start(mkd[:], mkd_d[:])
                nc.sync.dma_start(mkdT[:], mkdT_d[:])

            with ExitStack() as p1:
                p1sb = p1.enter_context(tc.tile_pool(name="p1sb", bufs=2))
                wpool = p1.enter_context(tc.tile_pool(name="wpool", bufs=1))
                p2sb = p1.enter_context(tc.tile_pool(name="p2sb", bufs=2))
                p2sb3 = p1.enter_context(tc.tile_pool(name="p2sb3", bufs=3))
                ptpool = p1.enter_context(tc.tile_pool(name="ptpool", bufs=4))
                mpool = p1.enter_context(tc.tile_pool(name="mpool", bufs=3))
                osb_p = p1.enter_context(tc.tile_pool(name="osbp", bufs=2))
                qkvps = p1.enter_context(tc.tile_pool(name="qkvps", bufs=2, space="PSUM"))
                p512 = p1.enter_context(tc.tile_pool(name="p512", bufs=2, space="PSUM"))
                cxops = p1.enter_context(tc.tile_pool(name="cxops", bufs=2, space="PSUM"))

                # w_qkv slices: split into per-h-tile DMAs so QKV starts early
                w_sb = wpool.tile([128, NT, 3 * HPC * D], F32R, tag="wqkv")
                for a in range(NT):
                    nc.gpsimd.dma_start(
                        w_sb[:, a, :],
                        wq_d[a * 128 : (a + 1) * 128, :],
                    )

                QW = 2 * HPC * D       # 512: q+k region width
                VW = HPC * D           # 256: v region width
                tmax_of = (lambda c: c * 4 + 3) if causal else (lambda c: NT - 1)

                def qkv_step(st):
                    s0 = st * 128
                    h_sb = p1sb.tile([128, NT, 128], F32R, tag="hT", bufs=2)
                    nc.gpsimd.dma_start(h_sb[:], hT_d[st])
                    qkv_ps = qkvps.tile([128, 3 * HPC * D], F32, tag="qkv")
                    for a in range(NT):
                        nc.tensor.matmul(
                            qkv_ps[:, 0:512], h_sb[:, a, :], w_sb[:, a, 0:512],
                            start=(a == 0), stop=(a == NT - 1),
                        )
                        nc.tensor.matmul(
                            qkv_ps[:, 512:768], h_sb[:, a, :], w_sb[:, a, 512:768],
                            start=(a == 0), stop=(a == NT - 1),
                        )
                    knat = p1sb.tile([128, VW], F32, tag="knat")
                    nc.any.tensor_copy(knat[:], qkv_ps[:, VW : 2 * VW])
                    nc.sync.dma_start(pk_d[s0 : s0 + 128, :], knat[:])
                    vnat = p1sb.tile([128, VW], F32, tag="vnat")
                    nc.any.tensor_copy(vnat[:], qkv_ps[:, 2 * VW : 3 * VW])
                    nc.sync.dma_start(pv_d[s0 : s0 + 128, :], vnat[:])
                    nc.vector.tensor_copy(v_all[:, st, :], qkv_ps[:, 2 * VW : 3 * VW])

                    # RoPE: roped = x*cosrep + swap(x)*sinsgn
                    base = qkv_ps[:]
                    tA = p1sb.tile([128, QW], F32, tag="ropeA")
                    tB = p1sb.tile([128, QW], F32, tag="ropeB")
                    roped = p1sb.tile([128, QW], F32, tag="roped")
                    cos_sb = p1sb.tile([128, 2 * D], F32, tag="cos")
                    sin_sb = p1sb.tile([128, 2 * D], F32, tag="sin")
                    nc.sync.dma_start(cos_sb[:], cos_d[s0 : s0 + 128, :])
                    nc.sync.dma_start(sin_sb[:], sin_d[s0 : s0 + 128, :])
                    cos_ap = _rep_ap(cos_sb[:], 2)
                    sin_ap = _rep_ap(sin_sb[:], 2)
                    nc.vector.tensor_mul(
                        tA[:].rearrange("p (a f) -> p a f", a=2, f=2 * D),
                        base[:, 0:QW].rearrange("p (a f) -> p a f", a=2, f=2 * D), cos_ap)
                    nc.vector.tensor_mul(
                        tB[:].rearrange("p (a f) -> p a f", a=2, f=2 * D),
                        _swap_ap(base, QW), sin_ap)
                    nc.vector.tensor_add(roped[:], tA[:], tB[:])
                    for j in range(2 * HPC):
                        t_ps = p512.tile([128, 128], F32, tag="p512", name=f"tr{st}_{j}")
                        nc.tensor.transpose(t_ps[:], roped[:, j * 128 : (j + 1) * 128], ident[:])
                        dst = qT[j] if j < HPC else kT[j - HPC]
                        nc.vector.tensor_copy(dst[:, s0 : s0 + 128], t_ps[:])

                def st_row(h, st):
                    # ST orientation row block: aw output + row sums
                    s0 = st * 128
                    ncols = ncols_of(st)
                    nchunk = (ncols + 511) // 512
                    pst = p2sb3.tile([128, S], F32, tag="pst")
                    sums = p2sb.tile([128, NCH], F32, tag="sums")
                    for c in range(nchunk):
                        n = min(512, ncols - c * 512)
                        pps = p512.tile([128, 512], F32, tag="p512", name=f"pps{h}_{st}_{c}")
                        nc.tensor.matmul(
                            pps[:, 0:n], qT[h][:, s0 : s0 + 128],
                            kT[h][:, c * 512 : c * 512 + n],
                            start=True, stop=True,
                        )
                        if causal and c == nchunk - 1:
                            off = ncols - 128 - c * 512
                            nc.vector.tensor_add(
                                pps[:, off : off + 128], pps[:, off : off + 128], mkd[:]
                            )
                        elif mask_mode == "full":
                            mt = mpool.tile([128, 512], F32, tag="mst")
                            nc.sync.dma_start(mt[:, 0:n], mk_d[s0 : s0 + 128, c * 512 : c * 512 + n])
                            nc.vector.tensor_add(pps[:, 0:n], pps[:, 0:n], mt[:, 0:n])
                        nc.scalar.activation(
                            pst[:, c * 512 : c * 512 + n], pps[:, 0:n], EXP,
                            scale=SCALE, accum_out=sums[:, c : c + 1],
                        )
                    tot = p2sb.tile([128, 1], F32, tag="tot")
                    if nchunk > 1:
                        nc.vector.reduce_sum(tot[:], sums[:, 0:nchunk], axis=mybir.AxisListType.X)
                    else:
                        nc.vector.tensor_copy(tot[:], sums[:, 0:1])
                    nc.vector.reciprocal(recip_all[h][:, st : st + 1], tot[:])
                    nc.sync.dma_start(aw_d[h, s0 : s0 + 128, 0:ncols], pst[:, 0:ncols])

                def t_burst(h, c):
                    # pT + AV for chunk c (software-pipelined by one t-step)
                    tmax = tmax_of(c)
                    cps = cxops.tile([128, 512], F32, tag="cx", name=f"ctx{h}_{c}")
                    pending = None
                    for t in range(tmax + 1):
                        t0 = t * 128
                        off = max(0, t0 - c * 512) if causal else 0
                        ptp = p512.tile([128, 512], F32, tag="p512", name=f"ptp{h}_{t}_{c}")
                        nc.tensor.matmul(
                            ptp[:, off:512], kT[h][:, t0 : t0 + 128],
                            qT[h][:, c * 512 + off : (c + 1) * 512],
                            start=True, stop=True,
                        )
                        if pending is not None:
                            pt, poff, ptsb = pending
                            nc.tensor.matmul(
                                cps[:, poff:512], v_all[:, pt, h * D : (h + 1) * D],
                                ptsb[:, poff:512],
                                start=(pt == 0), stop=False, skip_group_check=True,
                            )
                        if causal and t // 4 == c:
                            nc.vector.tensor_add(
                                ptp[:, off : off + 128], ptp[:, off : off + 128], mkdT[:]
                            )
                        elif mask_mode == "full":
                            mtT = mpool.tile([128, 512], F32, tag="mtT")
                            nc.sync.dma_start(mtT[:], mkT_d[t0 : t0 + 128, c * 512 : (c + 1) * 512])
                            nc.vector.tensor_add(ptp[:], ptp[:], mtT[:])
                        ptsb = ptpool.tile([128, 512], F32R, tag="ptsb", name=f"ptsb{h}_{t}_{c}")
                        nc.scalar.activation(ptsb[:, off:512], ptp[:, off:512], EXP, scale=SCALE)
                        pending = (t, off, ptsb)
                    pt, poff, ptsb = pending
                    nc.tensor.matmul(
                        cps[:, poff:512], v_all[:, pt, h * D : (h + 1) * D],
                        ptsb[:, poff:512],
                        start=(pt == 0), stop=True, skip_group_check=True,
                    )

                    # rbc for this chunk: rbc[p, s] = 1/rowsum[s]
                    rt_ps = p512.tile([16, 128], F32, tag="p512", name=f"rtps{h}_{c}")
                    nc.tensor.transpose(rt_ps[:], recip_all[h][:], ident[:])
                    rt16 = p2sb.tile([16, 128], F32R, tag="rt16")
                    nc.vector.tensor_copy(rt16[:], rt_ps[:])
                    rbc = p2sb.tile([128, 512], F32, tag="rbc")
                    for g in range(4):
                        st = 4 * c + g if causal else g  # noqa (causal always here)
                        sel_ap = AP(
                            sel_sb[:].tensor, sel_sb[:].offset + st,
                            [list(sel_sb[:].ap[0]), [0, 128]],
                        )
                        rb_ps = p512.tile([128, 128], F32, tag="p512", name=f"rbps{h}_{c}_{g}")
                        nc.tensor.matmul(
                            rb_ps[:], sel_ap, rt16[:],
                            start=True, stop=True,
                        )
                        nc.vector.tensor_copy(rbc[:, g * 128 : (g + 1) * 128], rb_ps[:])
                    nc.vector.tensor_mul(
                        ctxT[h][:, c * 512 : (c + 1) * 512], cps[:], rbc[:],
                    )

                def outproj_chunk(c):
                    # outp[hid, s-chunk] = sum_h sum_d woutT[d, hid] ctxT[d, s]
                    for ht in range(NT):
                        hh0 = ht * 128
                        o_ps = cxops.tile([128, 512], F32, tag="cx", name=f"o{ht}_{c}")
                        for h in range(HPC):
                            nc.tensor.matmul(
                                o_ps[:], wo_sb[:, h, hh0 : hh0 + 128],
                                ctxT[h][:, c * 512 : (c + 1) * 512],
                                start=(h == 0), stop=(h == HPC - 1),
                                skip_group_check=True,
                            )
                        osb = osb_p.tile([128, 512], F32, tag="osb")
                        nc.any.tensor_copy(osb[:], o_ps[:])
                        nc.sync.dma_start(
                            op_d[hh0 : hh0 + 128, c * 512 : (c + 1) * 512], osb[:]
                        )

                # ---------- fused schedule ----------
                for st in range(NT):
                    qkv_step(st)
                    if st == 0:
                        # late-load phase-2/3 weights so they don't delay QKV DMAs
                        nc.gpsimd.dma_start(wo_sb[:], wo_d.rearrange("(h p) f -> p h f", p=128))
                        nc.gpsimd.dma_start(sel_sb[:], sel_d[:])
                    for h in range(HPC):
                        st_row(h, st)
                    if causal and st % 4 == 3:
                        c = st // 4
                        for h in range(HPC):
                            t_burst(h, c)
                        outproj_chunk(c)
                if not causal:
                    for c in range(NCH):
                        for h in range(HPC):
                            t_burst(h, c)
                        outproj_chunk(c)
                for h in range(HPC):
                    nc.sync.dma_start(rec_d[h], recip_all[h][:])

    nc.compile()
    return nc


def _host_consts():
    inv_freq = 1.0 / (10000.0 ** (np.arange(0, D, 2, dtype=np.float64) / D))  # [64]
    ang = np.arange(S, dtype=np.float64)[:, None] * inv_freq[None, :]          # [S, 64]
    cos = np.cos(ang).astype(np.float32)
    sin = np.sin(ang).astype(np.float32)
    cosrep = np.tile(cos, (1, 4))                                              # [S, 256]
    sinsgn = np.concatenate([-sin, sin, -sin, sin], axis=1)                    # [S, 256]
    i = np.arange(128)
    maskd = np.where(i[:, None] >= i[None, :], 0.0, NEG_INF).astype(np.float32)
    maskdT = maskd.T.copy()
    sel16 = np.eye(16, dtype=np.float32)
    return cosrep, sinsgn, maskd, maskdT, sel16


def _detect_mode(attn_mask):
    if not np.any(attn_mask):
        return "none"
    i = np.arange(S)
    causal_ref = np.where(i[:, None] >= i[None, :], 0.0, np.float32(NEG_INF)).astype(np.float32)
    if np.array_equal(attn_mask, causal_ref):
        return "causal"
    return "full"


def kernel(hidden_states, attn_mask, w_qkv, w_out):
    hidden_states = np.ascontiguousarray(hidden_states, dtype=np.float32)
    attn_mask = np.ascontiguousarray(attn_mask, dtype=np.float32)
    w_qkv = np.ascontiguousarray(w_qkv, dtype=np.float32)
    w_out = np.ascontiguousarray(w_out, dtype=np.float32)

    mode = _detect_mode(attn_mask)
    if mode not in _BUILD_CACHE:
        _BUILD_CACHE[mode] = build(mode)
    nc = _BUILD_CACHE[mode]

    cosrep, sinsgn, maskd, maskdT, sel16 = _host_consts()
    # blocked transpose: hTc[st, p, a*128+j] = hidden[st*128+j, a*128+p]
    h4 = hidden_states[0].reshape(NT, 128, NT, 128)
    hTc = np.ascontiguousarray(h4.transpose(0, 3, 2, 1).reshape(NT, 128, S))

    in_maps = []
    for core in range(N_CORES):
        heads = [HPC * core + j for j in range(HPC)]
        d = np.arange(D)
        rows = np.concatenate(
            [base + d * HEADS + h for base in (0, QKV, 2 * QKV) for h in heads]
        )
        wqkvT = np.ascontiguousarray(w_qkv[rows, :].T)     # [HIDDEN, 768]
        cols = np.concatenate([d * HEADS + h for h in heads])
        woutT = np.ascontiguousarray(w_out[:, cols].T)     # [256, HIDDEN]
        m = {
            "hTc": hTc, "wqkvT": wqkvT, "woutT": woutT,
            "cosrep": cosrep, "sinsgn": sinsgn, "sel16": sel16,
        }
        if mode == "causal":
            m["maskd"] = maskd
            m["maskdT"] = maskdT
        if mode == "full":
            m["mask"] = attn_mask
            m["maskT"] = np.ascontiguousarray(attn_mask.T)
        in_maps.append(m)

    trace = bool(int(os.environ.get("KERNEL_TRACE", "0")))
    kwargs = {}
    if trace:
        import ntff_shim
        ntff_shim.install()
        kwargs = {"trace": True, "trace_cores": [0]}
    res = bass_utils.run_bass_kernel_spmd(nc, in_maps, core_ids=list(range(N_CORES)), **kwargs)
    kernel.last_exec_time_ns = res.exec_time_ns
    kernel.last_results = res

    attn_output_T = np.zeros((HIDDEN, S), dtype=np.float32)
    aw = np.empty((HEADS, S, S), dtype=np.float32)
    pk = np.empty((HEADS, S, D), dtype=np.float32)
    pv = np.empty((HEADS, S, D), dtype=np.float32)
    for core in range(N_CORES):
        r = res.results[core]
        attn_output_T += r["outp"]
        blk = aw[HPC * core : HPC * (core + 1)]
        blk[:] = r["aw"]
        rec = r["rec"]  # [HPC, 128, NT]
        for j in range(HPC):
            recip_s = rec[j].T.reshape(S)  # [st,p] -> s = st*128+p
            blk[j] *= recip_s[:, None]
        pk[HPC * core : HPC * (core + 1)] = r["pk"].reshape(S, HPC, D).transpose(1, 0, 2)
        pv[HPC * core : HPC * (core + 1)] = r["pv"].reshape(S, HPC, D).transpose(1, 0, 2)

    return (
        np.ascontiguousarray(attn_output_T.T)[None, :, :],
        aw[None, :, :, :],
        pk[None, :, :, :],
        pv[None, :, :, :],
    )
